# revision 1
# baseline (speedup 1.0000x reference)
"""PairwiseConv1D (valid 1D conv, NWC x WIO -> NWC) on 8 TRN2 NeuronCores.

Strategy:
  - Data-parallel over batch: B=32 -> 4 batches per core, kernel replicated.
  - Host feeds x transposed per batch ([C, L]) so the contraction dim C sits
    on SBUF partitions; no on-device transposes needed.
  - out.T[f, i] = sum_k w[k].T @ xT[:, i+k] computed as 7 accumulating
    matmuls per 512-wide output chunk (PSUM bank = 512 fp32).
  - f32r matmul mode: full PE rate at N>=256 (fp32_mode=HIGH single-pass).
    bf16 matmuls measured consistently ~13us/rep SLOWER on this silicon
    (three independent runs; ldw-opt elision did not recover it), and
    walrus rejects mixed f32r x bf16 operands — so the PE always streams
    f32r. x travels DRAM->SBUF as bf16 (half the read traffic) and the
    DVE upcasts each pass's slots to an f32r compute image one pass ahead
    (UPX). Measured rel err ~1.7e-3.
  - Raw-bass Block style with explicit semaphores: this toolchain's walrus
    codegen allows at most ONE sync-wait per instruction, so every wait is a
    standalone wait_ge on the consuming engine's queue.
  - Engines: SP = x loads (HWDGE), PE = matmuls, DVE = PSUM->SBUF copies,
    ACT = output stores (HWDGE).
"""

import ml_dtypes
import numpy as np

import concourse.bass as bass
import concourse.mybir as mybir
from concourse.bass_utils import run_bass_kernel_spmd

B, L, C, K, F = 32, 8192, 128, 7, 128
NCORES = 8
BPC = B // NCORES  # batches per core
LOUT = L - K + 1  # 8186
CHUNK = 512
NCHUNK = (LOUT + CHUNK - 1) // CHUNK  # 16, last chunk = 506
NT = BPC * NCHUNK  # total psum chunks per core
NPSUM = 8  # psum banks in rotation (all of PSUM)
XDMA = 4  # DMAs per batch x-load
XCOLS = L // XDMA
GRP = 8  # output chunks per store DMA (~1-2 MB writes amortize turnaround)
NGRPBUF = 2  # output group slots
NGRP = NCHUNK // GRP  # 2 groups per pass
ILV = 4  # chunks interleaved per weight sweep on PE

# dtypes: x lands in DRAM/SBUF as bf16 (halves read traffic) and is
# upcast on-device by the DVE to f32r for the fast LDW+MM path (bf16
# matmuls measured consistently ~13us/rep slower on this silicon);
# output stored fp32
XDT = "bf16"
WDT = "f32r"
ODT = "f32"
UPX = True

_nc = None


def _dt(s):
    return {"f32r": mybir.dt.float32r, "bf16": mybir.dt.bfloat16,
            "f32": mybir.dt.float32}[s]


def _np_dt(s):
    return {"f32r": np.float32, "bf16": ml_dtypes.bfloat16,
            "f32": np.float32}[s]


def _build(reps=1, detect_races=True, ILV=ILV, XDMA=XDMA, NPSUM=NPSUM, GRP=GRP,
           NGRPBUF=NGRPBUF, XDT=XDT, WDT=WDT, ODT=ODT, ALIGNED=False,
           DRAIN="dve", DMALITE=False, UPX=UPX):
    XCOLS = L // XDMA
    NGRP = NCHUNK // GRP
    f32 = mybir.dt.float32
    xdt, wdt, odt = _dt(XDT), _dt(WDT), _dt(ODT)
    assert not UPX or XDT == "bf16"
    nc = bass.Bass(detect_race_conditions=detect_races)
    xT = nc.dram_tensor("xT", [BPC, C, L], xdt, kind="ExternalInput")
    w = nc.dram_tensor("w", [K, C, F], wdt, kind="ExternalInput")
    outT = nc.dram_tensor("outT", [BPC, F, LOUT], odt, kind="ExternalOutput")

    G = reps * BPC  # total batch passes
    TT = G * NCHUNK  # total psum chunks

    from contextlib import ExitStack

    with ExitStack() as ctx:
        wsb = ctx.enter_context(nc.sbuf_tensor([C, K * F], wdt))
        xbufA = [
            ctx.enter_context(nc.sbuf_tensor(f"xa{i}", [C, L], xdt))
            for i in range(2)
        ]
        # optional shifted-by-one image so odd taps read 4B-aligned (bf16 x)
        xbufS = (
            [
                ctx.enter_context(nc.sbuf_tensor(f"xs{i}", [C, L], xdt))
                for i in range(2)
            ]
            if ALIGNED
            else None
        )
        # UPX: f32r compute images filled by DVE upcasts from the bf16
        # landing (the BIR verifier requires producers feeding f32r matmuls
        # to emit f32r-rounded output)
        xbufF = (
            [
                ctx.enter_context(
                    nc.sbuf_tensor(f"xf{i}", [C, L], mybir.dt.float32r)
                )
                for i in range(2)
            ]
            if UPX
            else None
        )
        obuf = ctx.enter_context(nc.sbuf_tensor([F, NGRPBUF * GRP * CHUNK], odt))
        psum = ctx.enter_context(nc.psum_tensor([F, NPSUM * CHUNK], f32))
        xfsem = ctx.enter_context(nc.semaphore(name="xfsem")) if UPX else None
        wsem = ctx.enter_context(nc.semaphore())
        # per-x-DMA-slot sems: counting one sem per slot makes waits safe
        # against out-of-order completion across HWDGE queues
        xsems = [
            ctx.enter_context(nc.semaphore(name=f"xsem{c}")) for c in range(XDMA)
        ]
        pe_sem = ctx.enter_context(nc.semaphore())
        dve_sem = ctx.enter_context(nc.semaphore())
        # per-output-group-slot sems, same reasoning
        osems = [
            ctx.enter_context(nc.semaphore(name=f"osem{s}")) for s in range(NGRPBUF)
        ]
        block = ctx.enter_context(nc.Block())

        XINC = 32 if ALIGNED else 16  # sem count per slot per pass

        def chunk_n(j):
            return CHUNK if j < NCHUNK - 1 else LOUT - (NCHUNK - 1) * CHUNK

        # number of x-DMA slots chunk j reads from
        def slots_needed(j):
            cols = min(L, (j + 1) * CHUNK + K - 1)
            return -(-cols // XCOLS)

        @block.sync
        def _(sync):
            # weights: [K, C, F] -> SBUF [C, (K F)]
            sync.dma_start(
                wsb[:, :], w.ap().rearrange("k c f -> c k f")
            ).then_inc(wsem, 16)
            for g in range(G):
                b = g % BPC
                if g >= 2:
                    if UPX:
                        # landing buffer g%2 consumed by the casts of pass g-2
                        sync.wait_ge(xfsem, 4 * (g - 1))
                    else:
                        # buffer g%2 must be fully consumed by PE (pass g-2)
                        sync.wait_ge(pe_sem, (g - 1) * NCHUNK)
                xa = xbufA[g % 2]
                # DMALITE: timing probe — same DMA count/sems, ~1/16 traffic
                xw = 128 if DMALITE else XCOLS
                for c in range(XDMA):
                    sync.dma_start(
                        xa[:, c * XCOLS : c * XCOLS + xw],
                        xT[b, :, c * XCOLS : c * XCOLS + xw],
                    ).then_inc(xsems[c], 16)
                    if ALIGNED:
                        # shifted image: column t holds x[t+1]
                        ncols = XCOLS if c < XDMA - 1 else XCOLS - 1
                        sync.dma_start(
                            xbufS[g % 2][:, c * XCOLS : c * XCOLS + ncols],
                            xT[b, :, c * XCOLS + 1 : c * XCOLS + 1 + ncols],
                        ).then_inc(xsems[c], 16)
            # leave all semaphores at 0 so the NEFF can be re-executed
            QT = TT // GRP  # total output groups
            for s in range(NGRPBUF):
                sync.wait_ge(osems[s], 16 * (QT // NGRPBUF))
            for s in [wsem, pe_sem, dve_sem] + xsems + osems + (
                [xfsem] if UPX else []
            ):
                sync.sem_clear(s)

        ilv = ILV  # chunks interleaved per weight sweep

        @block.tensor
        def _(tensor):
            tensor.wait_ge(wsem, 16)
            xseen = [0] * XDMA
            xfseen = 0
            for g in range(G):
                xa = xbufF[g % 2] if UPX else xbufA[g % 2]
                xs = xbufS[g % 2] if ALIGNED else None
                # chunk quads, k-outer within a quad: 4 PSUM banks busy,
                # the other 4 stay free for the DVE drain
                for m in range(NCHUNK // ilv):
                    js = [ilv * m + i for i in range(ilv)]
                    ts = [g * NCHUNK + j for j in js]
                    if UPX:
                        need = 4 * g + slots_needed(js[-1])
                        if xfseen < need:
                            tensor.wait_ge(xfsem, need)
                            xfseen = need
                    else:
                        need = XINC * (g + 1)
                        for c in range(slots_needed(js[-1])):
                            if xseen[c] < need:
                                tensor.wait_ge(xsems[c], need)
                                xseen[c] = need
                    if ts[-1] >= NPSUM:
                        tensor.wait_ge(dve_sem, ts[-1] - NPSUM + 1)
                    ns = [chunk_n(j) for j in js]
                    pss = [
                        psum[:, (t % NPSUM) * CHUNK : (t % NPSUM) * CHUNK + n]
                        for t, n in zip(ts, ns)
                    ]
                    for k in range(K):
                        if ALIGNED and (k % 2 == 1):
                            xb, off = xs, k - 1
                        else:
                            xb, off = xa, k
                        for i in range(ilv):
                            rhs = xb[:, js[i] * CHUNK + off : js[i] * CHUNK + off + ns[i]]
                            ins = nc.tensor.matmul(
                                pss[i],
                                wsb[:, k * F : (k + 1) * F],
                                rhs,
                                start=(k == 0),
                                stop=(k == K - 1),
                                skip_group_check=True,
                            )
                    ins.then_inc(pe_sem, ilv)

        def upcast_pass(eng, g):
            # upcast pass g's landing slots into the fp32 compute image
            for c in range(XDMA):
                eng.wait_ge(xsems[c], 16 * (g + 1))
                nc.vector.tensor_copy(
                    xbufF[g % 2][:, c * XCOLS : (c + 1) * XCOLS],
                    xbufA[g % 2][:, c * XCOLS : (c + 1) * XCOLS],
                ).then_inc(xfsem, 1)

        def drain_body(eng, copy_op, do_stores):
            # drain two adjacent PSUM banks per copy (contiguous columns)
            for p in range(TT // 2):
                t = 2 * p
                if UPX and t % NCHUNK == 0:
                    g = t // NCHUNK
                    if g == 0:
                        upcast_pass(eng, 0)
                    if g + 1 < G:
                        upcast_pass(eng, g + 1)
                j = t % NCHUNK
                n = chunk_n(j) + chunk_n(j + 1)
                q = t // GRP  # output group
                s = q % NGRPBUF  # group slot
                eng.wait_ge(pe_sem, t + 2)
                if t % GRP == 0 and q >= NGRPBUF:
                    # group slot's previous store DMA must be done
                    eng.wait_ge(osems[s], 16 * (q // NGRPBUF))
                copy_op(
                    obuf[:, s * GRP * CHUNK + (t % GRP) * CHUNK :
                         s * GRP * CHUNK + (t % GRP) * CHUNK + n],
                    psum[:, (t % NPSUM) * CHUNK : (t % NPSUM) * CHUNK + n],
                ).then_inc(dve_sem, 2)
                if do_stores and (t + 2) % GRP == 0:
                    # last pair of group q drained (program order): store it
                    b = (q // NGRP) % BPC
                    cols0 = (q % NGRP) * GRP * CHUNK
                    ncols = min(GRP * CHUNK, LOUT - cols0)
                    eng.dma_start(
                        outT[b, :, cols0 : cols0 + ncols],
                        obuf[:, s * GRP * CHUNK : s * GRP * CHUNK + ncols],
                    ).then_inc(osems[s], 16)

        if DRAIN == "dve":

            @block.vector
            def _(vector):
                drain_body(vector, nc.vector.tensor_copy, do_stores=False)

            @block.scalar
            def _(scalar):
                QT = TT // GRP
                for q in range(QT):
                    b = (q // NGRP) % BPC
                    qq = q % NGRP  # group within pass
                    cols0 = qq * GRP * CHUNK
                    ncols = min(GRP * CHUNK, LOUT - cols0)
                    s = q % NGRPBUF
                    if DMALITE:
                        ncols = 128
                    scalar.wait_ge(dve_sem, (q + 1) * GRP)
                    scalar.dma_start(
                        outT[b, :, cols0 : cols0 + ncols],
                        obuf[:, s * GRP * CHUNK : s * GRP * CHUNK + ncols],
                    ).then_inc(osems[s], 16)

        else:  # DRAIN == "act": copies + stores all on the Activation engine

            @block.scalar
            def _(scalar):
                drain_body(scalar, nc.scalar.copy, do_stores=True)

    return nc


def kernel(x, kernel):
    global _nc
    x = np.asarray(x, dtype=np.float32)
    w = np.ascontiguousarray(
        np.asarray(kernel, dtype=np.float32).astype(_np_dt(WDT))
    )
    # [B, L, C, 1] -> per-batch transposed [B, C, L]
    xT = np.ascontiguousarray(
        np.transpose(x[..., 0], (0, 2, 1)).astype(_np_dt(XDT))
    )
    in_maps = [
        {"xT": xT[i * BPC : (i + 1) * BPC], "w": w} for i in range(NCORES)
    ]
    if _nc is None:
        _nc = _build()
    res = run_bass_kernel_spmd(_nc, in_maps, core_ids=list(range(NCORES)))
    outT = np.concatenate([r["outT"] for r in res.results], axis=0)  # [B,F,LOUT]
    out = np.transpose(outT.astype(np.float32), (0, 2, 1))[..., None]
    return np.ascontiguousarray(out)



# revision 19
# speedup vs baseline: 1.0053x; 1.0053x over previous
"""PairwiseConv1D (valid 1D conv, NWC x WIO -> NWC) on 8 TRN2 NeuronCores.

Strategy:
  - Data-parallel over batch: B=32 -> 4 batches per core, kernel replicated.
  - Host feeds x transposed per batch ([C, L]) so the contraction dim C sits
    on SBUF partitions; no on-device transposes needed.
  - out.T[f, i] = sum_k w[k].T @ xT[:, i+k] computed as 7 accumulating
    matmuls per 512-wide output chunk (PSUM bank = 512 fp32).
  - f32r matmul mode: full PE rate at N>=256. walrus rejects mixed
    f32r x bf16 operands, so x travels DRAM->SBUF as bf16 (half the read
    traffic) and the DVE upcasts each pass's slots to an f32r compute
    image one pass ahead (UPX). Output stored bf16, upcast on host
    (rel err ~2.4e-3 total, gate is 2e-2).
  - Raw-bass Block style with explicit semaphores: this toolchain's walrus
    codegen allows at most ONE sync-wait per instruction, so every wait is a
    standalone wait_ge on the consuming engine's queue.
  - Engines: SP = x loads (HWDGE), PE = matmuls, DVE = PSUM->SBUF copies,
    ACT = output stores (HWDGE).

Measured facts on this silicon (probe batches, robust 17/97-rep diff
timing, +-1.5us):
  - Per-rep time is purely proportional to streamed matmul rows:
    doubling the stream doubles time (235.6us vs 117.8us). Effective
    sustained rate ~514ps/row (~1.94GHz, not the 2.4GHz peak), so the
    kernel sits AT the PE streaming floor.
  - No measurable cost from: LDWEIGHTS (walrus --enable-ldw-opt=true
    elides 448->112, zero time change), PE sem-waits (coarser xfsem
    granularity: no change), DMA volume (DMALITE probe: no change),
    bf16 vs f32r stream rate (equal).
  - Non-PE floor is ~80-82us (KPROBE=1/4 probes) — DVE drains+upcasts;
    invisible while PE-bound at ~118.
  - GPSIMD (Pool) tensor_copy is ~6x slower than the cost model says;
    upcasts-on-Pool nearly binds (~93-112us). Kept on DVE.
  - fp8 DoubleRow (0.5 cyc/row) would land ~80-85us but rel err ~2.5e-2
    exceeds the 2e-2 gate. Winograd loses to transpose+drain overheads.
"""

import ml_dtypes
import numpy as np

import concourse.bass as bass
import concourse.mybir as mybir
from concourse.bass_utils import run_bass_kernel_spmd

B, L, C, K, F = 32, 8192, 128, 7, 128
NCORES = 8
BPC = B // NCORES  # batches per core
LOUT = L - K + 1  # 8186
CHUNK = 512
NCHUNK = (LOUT + CHUNK - 1) // CHUNK  # 16, last chunk = 506
NT = BPC * NCHUNK  # total psum chunks per core
NPSUM = 8  # psum banks in rotation (all of PSUM)
XDMA = 4  # DMAs per batch x-load
XCOLS = L // XDMA
GRP = 8  # output chunks per store DMA (~1-2 MB writes amortize turnaround)
NGRPBUF = 2  # output group slots
NGRP = NCHUNK // GRP  # 2 groups per pass
ILV = 4  # chunks interleaved per weight sweep on PE

# dtypes: x lands in DRAM/SBUF as bf16 (halves read traffic) and is
# upcast on-device by the DVE to f32r for the fast LDW+MM path; output
# stored bf16 (halves write traffic; host upcasts to fp32 — rel err
# ~2.4e-3 total, well under the 2e-2 gate)
XDT = "bf16"
WDT = "f32r"
ODT = "bf16"
UPX = True

_nc = None


def _dt(s):
    return {"f32r": mybir.dt.float32r, "bf16": mybir.dt.bfloat16,
            "f32": mybir.dt.float32}[s]


def _np_dt(s):
    return {"f32r": np.float32, "bf16": ml_dtypes.bfloat16,
            "f32": np.float32}[s]


def _build(reps=1, detect_races=True, ILV=ILV, XDMA=XDMA, NPSUM=NPSUM, GRP=GRP,
           NGRPBUF=NGRPBUF, XDT=XDT, WDT=WDT, ODT=ODT, ALIGNED=False,
           DRAIN="dve", DMALITE=False, UPX=UPX, CHUNK=CHUNK, KPROBE=None,
           WSAME=False, UPENG="dve", SALT=0, NOXW=False, NODW=False,
           XFGRAN="slot", WAIT0=False, KDOUBLE=False):
    XCOLS = L // XDMA
    NCHUNK = (LOUT + CHUNK - 1) // CHUNK
    NT = BPC * NCHUNK
    NGRP = NCHUNK // GRP
    f32 = mybir.dt.float32
    xdt, wdt, odt = _dt(XDT), _dt(WDT), _dt(ODT)
    assert not UPX or XDT == "bf16"
    nc = bass.Bass(detect_race_conditions=detect_races)
    xT = nc.dram_tensor("xT", [BPC, C, L], xdt, kind="ExternalInput")
    w = nc.dram_tensor("w", [K, C, F], wdt, kind="ExternalInput")
    outT = nc.dram_tensor("outT", [BPC, F, LOUT], odt, kind="ExternalOutput")

    G = reps * BPC  # total batch passes
    TT = G * NCHUNK  # total psum chunks

    from contextlib import ExitStack

    with ExitStack() as ctx:
        if SALT:
            # unused; only perturbs the BIR/HLO hash to bust the NEFF cache
            ctx.enter_context(nc.sbuf_tensor("salt", [1, SALT], f32))
        wsb = ctx.enter_context(nc.sbuf_tensor([C, K * F], wdt))
        xbufA = [
            ctx.enter_context(nc.sbuf_tensor(f"xa{i}", [C, L], xdt))
            for i in range(2)
        ]
        # optional shifted-by-one image so odd taps read 4B-aligned (bf16 x)
        xbufS = (
            [
                ctx.enter_context(nc.sbuf_tensor(f"xs{i}", [C, L], xdt))
                for i in range(2)
            ]
            if ALIGNED
            else None
        )
        # UPX: f32r compute images filled by DVE upcasts from the bf16
        # landing (the BIR verifier requires producers feeding f32r matmuls
        # to emit f32r-rounded output)
        xbufF = (
            [
                ctx.enter_context(
                    nc.sbuf_tensor(f"xf{i}", [C, L], mybir.dt.float32r)
                )
                for i in range(2)
            ]
            if UPX
            else None
        )
        obuf = ctx.enter_context(nc.sbuf_tensor([F, NGRPBUF * GRP * CHUNK], odt))
        psum = ctx.enter_context(nc.psum_tensor([F, NPSUM * CHUNK], f32))
        xfsem = ctx.enter_context(nc.semaphore(name="xfsem")) if UPX else None
        wsem = ctx.enter_context(nc.semaphore())
        # per-x-DMA-slot sems: counting one sem per slot makes waits safe
        # against out-of-order completion across HWDGE queues
        xsems = [
            ctx.enter_context(nc.semaphore(name=f"xsem{c}")) for c in range(XDMA)
        ]
        pe_sem = ctx.enter_context(nc.semaphore())
        dve_sem = ctx.enter_context(nc.semaphore())
        # per-output-group-slot sems, same reasoning
        osems = [
            ctx.enter_context(nc.semaphore(name=f"osem{s}")) for s in range(NGRPBUF)
        ]
        block = ctx.enter_context(nc.Block())

        XINC = 32 if ALIGNED else 16  # sem count per slot per pass

        def chunk_n(j):
            return CHUNK if j < NCHUNK - 1 else LOUT - (NCHUNK - 1) * CHUNK

        # number of x-DMA slots chunk j reads from
        def slots_needed(j):
            cols = min(L, (j + 1) * CHUNK + K - 1)
            return -(-cols // XCOLS)

        @block.sync
        def _(sync):
            # weights: [K, C, F] -> SBUF [C, (K F)]
            sync.dma_start(
                wsb[:, :], w.ap().rearrange("k c f -> c k f")
            ).then_inc(wsem, 16)
            for g in range(G):
                b = g % BPC
                if g >= 2:
                    if UPX:
                        # landing buffer g%2 consumed by the casts of pass g-2
                        sync.wait_ge(
                            xfsem,
                            (g - 1) if XFGRAN == "pass" else 4 * (g - 1),
                        )
                    else:
                        # buffer g%2 must be fully consumed by PE (pass g-2)
                        sync.wait_ge(pe_sem, (g - 1) * NCHUNK)
                xa = xbufA[g % 2]
                # DMALITE: timing probe — same DMA count/sems, ~1/16 traffic
                xw = 128 if DMALITE else XCOLS
                for c in range(XDMA):
                    sync.dma_start(
                        xa[:, c * XCOLS : c * XCOLS + xw],
                        xT[b, :, c * XCOLS : c * XCOLS + xw],
                    ).then_inc(xsems[c], 16)
                    if ALIGNED:
                        # shifted image: column t holds x[t+1]
                        ncols = XCOLS if c < XDMA - 1 else XCOLS - 1
                        sync.dma_start(
                            xbufS[g % 2][:, c * XCOLS : c * XCOLS + ncols],
                            xT[b, :, c * XCOLS + 1 : c * XCOLS + 1 + ncols],
                        ).then_inc(xsems[c], 16)
            # leave all semaphores at 0 so the NEFF can be re-executed
            QT = TT // GRP  # total output groups
            for s in range(NGRPBUF):
                sync.wait_ge(osems[s], 16 * (QT // NGRPBUF))
            for s in [wsem, pe_sem, dve_sem] + xsems + osems + (
                [xfsem] if UPX else []
            ):
                sync.sem_clear(s)

        ilv = ILV  # chunks interleaved per weight sweep

        @block.tensor
        def _(tensor):
            tensor.wait_ge(wsem, 16)
            xseen = [0] * XDMA
            xfseen = 0
            for g in range(G):
                xa = xbufF[g % 2] if UPX else xbufA[g % 2]
                xs = xbufS[g % 2] if ALIGNED else None
                # chunk quads, k-outer within a quad: 4 PSUM banks busy,
                # the other 4 stay free for the DVE drain
                for m in range(NCHUNK // ilv):
                    js = [ilv * m + i for i in range(ilv)]
                    ts = [g * NCHUNK + j for j in js]
                    z = 0 if WAIT0 else 1  # WAIT0: emit waits, threshold 0
                    if UPX:
                        need = (g + 1 if XFGRAN == "pass"
                                else 4 * g + slots_needed(js[-1]))
                        if xfseen < need and not NOXW:
                            tensor.wait_ge(xfsem, need * z)
                            xfseen = need
                    else:
                        need = XINC * (g + 1)
                        for c in range(slots_needed(js[-1])):
                            if xseen[c] < need and not NOXW:
                                tensor.wait_ge(xsems[c], need * z)
                                xseen[c] = need
                    if ts[-1] >= NPSUM and not NODW:
                        tensor.wait_ge(dve_sem, (ts[-1] - NPSUM + 1) * z)
                    ns = [chunk_n(j) for j in js]
                    pss = [
                        psum[:, (t % NPSUM) * CHUNK : (t % NPSUM) * CHUNK + n]
                        for t, n in zip(ts, ns)
                    ]
                    KP = KPROBE or K
                    NS = 2 if KDOUBLE else 1  # timing probe: 2x the stream
                    for s2 in range(NS):
                        for k in range(KP):
                            if ALIGNED and (k % 2 == 1):
                                xb, off = xs, k - 1
                            else:
                                xb, off = xa, k
                            kw = 0 if WSAME else k
                            for i in range(ilv):
                                rhs = xb[:, js[i] * CHUNK + off : js[i] * CHUNK + off + ns[i]]
                                ins = nc.tensor.matmul(
                                    pss[i],
                                    wsb[:, kw * F : (kw + 1) * F],
                                    rhs,
                                    start=(k == 0 and s2 == 0),
                                    stop=(k == KP - 1 and s2 == NS - 1),
                                    skip_group_check=True,
                                )
                    ins.then_inc(pe_sem, ilv)

        def upcast_pass(eng, g):
            # upcast pass g's landing slots into the fp32 compute image
            for c in range(XDMA):
                eng.wait_ge(xsems[c], 16 * (g + 1))
                ins = nc.vector.tensor_copy(
                    xbufF[g % 2][:, c * XCOLS : (c + 1) * XCOLS],
                    xbufA[g % 2][:, c * XCOLS : (c + 1) * XCOLS],
                )
                if XFGRAN == "slot":
                    ins.then_inc(xfsem, 1)
                elif c == XDMA - 1:
                    ins.then_inc(xfsem, 1)

        def drain_body(eng, copy_op, do_stores):
            # drain two adjacent PSUM banks per copy (contiguous columns)
            for p in range(TT // 2):
                t = 2 * p
                if UPX and UPENG == "dve" and t % NCHUNK == 0:
                    g = t // NCHUNK
                    if g == 0:
                        upcast_pass(eng, 0)
                    if g + 1 < G:
                        upcast_pass(eng, g + 1)
                j = t % NCHUNK
                n = chunk_n(j) + chunk_n(j + 1)
                q = t // GRP  # output group
                s = q % NGRPBUF  # group slot
                eng.wait_ge(pe_sem, t + 2)
                if t % GRP == 0 and q >= NGRPBUF:
                    # group slot's previous store DMA must be done
                    eng.wait_ge(osems[s], 16 * (q // NGRPBUF))
                copy_op(
                    obuf[:, s * GRP * CHUNK + (t % GRP) * CHUNK :
                         s * GRP * CHUNK + (t % GRP) * CHUNK + n],
                    psum[:, (t % NPSUM) * CHUNK : (t % NPSUM) * CHUNK + n],
                ).then_inc(dve_sem, 2)
                if do_stores and (t + 2) % GRP == 0:
                    # last pair of group q drained (program order): store it
                    b = (q // NGRP) % BPC
                    cols0 = (q % NGRP) * GRP * CHUNK
                    ncols = min(GRP * CHUNK, LOUT - cols0)
                    eng.dma_start(
                        outT[b, :, cols0 : cols0 + ncols],
                        obuf[:, s * GRP * CHUNK : s * GRP * CHUNK + ncols],
                    ).then_inc(osems[s], 16)

        if UPX and UPENG == "pool":
            # upcasts on the (otherwise idle) Pool/GPSIMD engine
            @block.gpsimd
            def _(g_eng):
                for g in range(G):
                    if g >= 2:
                        # xbufF slot g%2 still read by PE pass g-2
                        g_eng.wait_ge(pe_sem, (g - 1) * NCHUNK)
                    for c in range(XDMA):
                        g_eng.wait_ge(xsems[c], 16 * (g + 1))
                        ins = nc.gpsimd.tensor_copy(
                            xbufF[g % 2][:, c * XCOLS : (c + 1) * XCOLS],
                            xbufA[g % 2][:, c * XCOLS : (c + 1) * XCOLS],
                        )
                        if XFGRAN == "slot" or c == XDMA - 1:
                            ins.then_inc(xfsem, 1)

        if DRAIN == "dve":

            @block.vector
            def _(vector):
                drain_body(vector, nc.vector.tensor_copy, do_stores=False)

            @block.scalar
            def _(scalar):
                QT = TT // GRP
                for q in range(QT):
                    b = (q // NGRP) % BPC
                    qq = q % NGRP  # group within pass
                    cols0 = qq * GRP * CHUNK
                    ncols = min(GRP * CHUNK, LOUT - cols0)
                    s = q % NGRPBUF
                    if DMALITE:
                        ncols = 128
                    scalar.wait_ge(dve_sem, (q + 1) * GRP)
                    scalar.dma_start(
                        outT[b, :, cols0 : cols0 + ncols],
                        obuf[:, s * GRP * CHUNK : s * GRP * CHUNK + ncols],
                    ).then_inc(osems[s], 16)

        else:  # DRAIN == "act": copies + stores all on the Activation engine

            @block.scalar
            def _(scalar):
                drain_body(scalar, nc.scalar.copy, do_stores=True)

    return nc


def kernel(x, kernel):
    global _nc
    x = np.asarray(x, dtype=np.float32)
    w = np.ascontiguousarray(
        np.asarray(kernel, dtype=np.float32).astype(_np_dt(WDT))
    )
    # [B, L, C, 1] -> per-batch transposed [B, C, L]
    xT = np.ascontiguousarray(
        np.transpose(x[..., 0], (0, 2, 1)).astype(_np_dt(XDT))
    )
    in_maps = [
        {"xT": xT[i * BPC : (i + 1) * BPC], "w": w} for i in range(NCORES)
    ]
    if _nc is None:
        _nc = _build()
    res = run_bass_kernel_spmd(_nc, in_maps, core_ids=list(range(NCORES)))
    outT = np.concatenate([r["outT"] for r in res.results], axis=0)  # [B,F,LOUT]
    out = np.transpose(outT.astype(np.float32), (0, 2, 1))[..., None]
    return np.ascontiguousarray(out)

